# revision 29
# baseline (speedup 1.0000x reference)
"""GCNCheb Trainium2 kernel: out[b,n,fo] = sum_k T_k[b,n,:] @ W[k] + bias.

T_k recurrence (matrix powers P_j = L^j x with T0=P0, T1=P1, Tk=2*P_k - T_{k-2})
is linear, so the K/F_in contraction is re-expressed over pure powers with
host-precombined weights V_j:
    out = P0 (W0-W2) + P1 (W1-W3) + P2 (2 W2) + P3 (2 W3) + bias

Distribution over 8 NeuronCores: 1D row-shard of L. Core r holds the column
slice Lc_r = L[:, r*1024:(r+1)*1024] (== L[rows_r,:].T since L is symmetric),
pre-tiled on host to [4, 128, 64, 256] (quarter-of-m-shard major). X is
[N, B*F_in] = [8192, 128] (batch folded into columns), tiled to [128, 64, 128].
L is bf16 and fully SBUF-resident (16.8 MB), streamed on the scalar queue in
few large chunks; the sync queue carries only small latency-critical
transfers so nothing queues behind megabytes.

Exchange (the critical path): each step's row-shard is exchanged with TWO
half-shard AllGathers in fp8 e3m4 — half the wire bytes of bf16, and the
512 KB gathered size stays under the ~1 MB Mesh->RDH algorithm crossover.
The matmul path stays bf16: each phase's PSUM result is copied out twice
(bf16 for local transposes, fp8 for the wire) and gathered halves are
up-cast fp8->bf16 by the idle vector engine right after each return DMA.
Step 2 consumes gathered k-tiles half-arrival-major and computes its first
output m-half before the second so its gathers trigger early. Step 3
accumulates both output column halves in two interleaved PSUM chains (one
stationary X2 k-tile feeds both 512-wide matmuls), so the work after the
final gather is one short pass; the projection contracts P_j^T tiles with
block-diagonal weights and the out^T shard leaves in bf16; the host
untangles layout and adds bias in fp32.
"""

import os
import sys

sys.path.insert(0, "/opt/trn_rl_repo")

import numpy as np

import concourse.bass as bass
import concourse.mybir as mybir
import concourse.tile as tile
from concourse import bacc, bass_utils
from concourse.masks import make_identity

B, N, F_IN, F_OUT, K = 4, 8192, 32, 64, 4
NCORES = 8
P = 128
SH = N // NCORES          # rows per core (1024)
BF = B * F_IN             # folded X columns (128)
KT = N // P               # contraction tiles (64)
MT = SH // P              # output row tiles per core (8)
MH = MT // 2              # half-shard m-tiles (4)
QH = 2                    # output halves: (b in {2h, 2h+1}) x F_OUT = 128 partitions
SH4 = SH // 4

VARIANT = os.environ.get("GCN_VARIANT", "bf16")

_DT = {"bf16": mybir.dt.bfloat16, "fp32": mybir.dt.float32}


def _np_dt(variant):
    if variant == "bf16":
        import ml_dtypes

        return np.dtype(ml_dtypes.bfloat16)
    return np.dtype(np.float32)


def build_nc(variant=VARIANT):
    dt = _DT[variant]
    f32 = mybir.dt.float32
    f8 = mybir.dt.float8e3  # e3m4: 4 mantissa bits, range +-15.5 >> max|X|~4

    nc = bacc.Bacc()
    # all pre-tiled on host: partition-major, fully contiguous per partition
    Lc = nc.dram_tensor("Lc", [4, P, KT, SH4], dt, kind="ExternalInput")
    X0 = nc.dram_tensor("X0", [P, KT, BF], dt, kind="ExternalInput")
    X0T = nc.dram_tensor("X0T", [BF, SH], dt, kind="ExternalInput")
    WH = nc.dram_tensor("WH", [P, K, QH, P], dt, kind="ExternalInput")
    OUT = nc.dram_tensor("OUT", [QH, P, SH], dt, kind="ExternalOutput")

    # half h of a step's gather delivers m-tiles 4h..4h+3 of every rank,
    # i.e. k-tiles {r*MT + 4h + m}
    def half_kts(h):
        return [r * MT + 4 * h + m for r in range(NCORES) for m in range(MH)]

    with tile.TileContext(nc) as tc:
        with (
            tc.tile_pool(name="lres", bufs=1) as lres_pool,
            tc.tile_pool(name="xbuf", bufs=2) as x_pool,
            tc.tile_pool(name="ybuf", bufs=2) as y_pool,
            tc.tile_pool(name="y8buf", bufs=2) as y8_pool,
            tc.tile_pool(name="proj", bufs=1) as proj_pool,
            tc.tile_pool(name="psum", bufs=1, space="PSUM") as psum_pool,
            tc.tile_pool(name="dram", bufs=1, space="DRAM") as dram_pool,
        ):
            # --- dummy 256 B AllGather, triggerable at t~0: it becomes the
            # runtime's first-collective rendezvous, absorbing the ~50 us
            # barrier, the ncfw first-pickup latency and the cross-core
            # start skew while step 1 is still streaming L. The real
            # exchanges then run at pure wire latency. ---
            dsb = proj_pool.tile([1, 64], f32, tag="dsb")
            nc.gpsimd.memset(dsb[:], 0.0)
            dummy_in = dram_pool.tile([1, 64], f32, name="dummy_in")
            dummy_out = dram_pool.tile(
                [NCORES, 64], f32, addr_space="Shared", name="dummy_out"
            )
            nc.sync.dma_start(dummy_in.opt(), dsb[:])
            nc.gpsimd.collective_compute(
                "AllGather",
                mybir.AluOpType.bypass,
                replica_groups=[list(range(NCORES))],
                ins=[dummy_in.opt()],
                outs=[dummy_out.opt()],
            )

            # identity for PE transposes: first on the gpsimd queue
            ident = proj_pool.tile([P, P], dt, tag="ident")
            make_identity(nc, ident[:])

            # --- sync queue: small latency-critical loads only ---
            x_cur = x_pool.tile([P, KT, BF], dt, tag="x", name="x0")
            nc.sync.dma_start(x_cur[:, :2, :], X0[:, :2, :])
            nc.sync.dma_start(x_cur[:, 2:8, :], X0[:, 2:8, :])
            whs = proj_pool.tile([P, K, QH, P], dt, tag="whs")
            nc.sync.dma_start(whs[:], WH[:, :, :, :])
            pt0 = proj_pool.tile([P, SH], dt, tag="pt0")
            nc.sync.dma_start(pt0[:], X0T[:, :])

            # --- scalar queue: X0 bulk, then the Lc stream in large
            # chunks (progressively sized so step 1 starts immediately) ---
            nc.scalar.dma_start(x_cur[:, 8:, :], X0[:, 8:, :])
            lc_res = lres_pool.tile([P, 4, KT, SH4], dt, tag="lc_res")
            lc_chunks = {
                0: [(0, 2), (2, 8), (8, 24), (24, 64)],
                1: [(0, 32), (32, 64)],
                2: [(0, 64)],
                3: [(0, 64)],
            }
            for q in range(4):
                for ko0, ko1 in lc_chunks[q]:
                    nc.scalar.dma_start(
                        lc_res[:, q, ko0:ko1, :], Lc[q, :, ko0:ko1, :]
                    )

            pt = [pt0, None, None, None]
            out_sb = proj_pool.tile([P, QH, 2, 512], dt, tag="out_sb")

            def lhsT_res(kt, mt):
                q, m = divmod(mt, 2)
                return lc_res[:, q, kt, m * P : (m + 1) * P]

            def gather_half(step, h, y8, x_nxt):
                """AllGather an fp8 m-half of the shard.

                The fp8->bf16 up-cast happens INSIDE the return DMA (SWDGE
                dtype-cast) to keep long-waiting cast instructions off the
                strict-FIFO vector queue.
                """
                shard = dram_pool.tile([P, MH, BF], f8, name=f"shard{step}_{h}")
                full = dram_pool.tile(
                    [NCORES * P, MH, BF],
                    f8,
                    addr_space="Shared",
                    name=f"full{step}_{h}",
                )
                nc.sync.dma_start(shard.opt(), y8[:, 4 * h : 4 * h + MH, :])
                nc.gpsimd.collective_compute(
                    "AllGather",
                    mybir.AluOpType.bypass,
                    replica_groups=[list(range(NCORES))],
                    ins=[shard.opt()],
                    outs=[full.opt()],
                )
                xv = x_nxt[:].rearrange("p (r mt) f -> p r mt f", r=NCORES)
                nc.gpsimd.dma_start(
                    xv[:, :, 4 * h : 4 * h + MH, :],
                    full[:].rearrange("(r p) mt f -> p r mt f", p=P),
                )

            def transposes(step, yshd, mts):
                """PE-transpose row-shard m-tiles into P_j^T for projection."""
                if pt[step] is None:
                    pt[step] = proj_pool.tile(
                        [P, SH], dt, tag=f"pt{step}", name=f"pt{step}"
                    )
                for mt in mts:
                    tp = psum_pool.tile(
                        [P, P], dt, tag=f"ps{mt}", name=f"tp{step}_{mt}"
                    )
                    nc.tensor.transpose(tp[:], yshd[:, mt, :], ident[:])
                    nc.vector.tensor_copy(
                        pt[step][:, mt * P : (mt + 1) * P], tp[:]
                    )

            # ---- step 1: m-quarter phases track the Lc quarter arrival
            # order; halves gather as soon as their phases complete ----
            yshd1 = y_pool.tile([P, MT, BF], dt, tag="yshd", name="yshd1")
            y81 = y8_pool.tile([P, MT, BF], f8, tag="y8", name="y81")
            x1 = x_pool.tile([P, KT, BF], dt, tag="x", name="x1")
            ypsum = {
                mt: psum_pool.tile([P, BF], f32, tag=f"ps{mt}", name=f"y1_{mt}")
                for mt in range(MT)
            }
            for ph in range(4):
                mts = (2 * ph, 2 * ph + 1)
                for kt in range(KT):
                    for mt in mts:
                        nc.tensor.matmul(
                            ypsum[mt][:],
                            lhsT=lhsT_res(kt, mt),
                            rhs=x_cur[:, kt, :],
                            start=(kt == 0),
                            stop=(kt == KT - 1),
                        )
                for mt in mts:
                    nc.vector.tensor_copy(yshd1[:, mt, :], ypsum[mt][:])
                    nc.vector.tensor_copy(y81[:, mt, :], ypsum[mt][:])
                if ph % 2 == 1:
                    gather_half(1, ph // 2, y81, x1)
            # pt1 transposes run in the PE hole while the barrier/AG1 fly
            transposes(1, yshd1, range(MT))

            # ---- step 2: both X1 halves feed output m-half 0-3 first so
            # its gather triggers as early as possible ----
            yshd2 = y_pool.tile([P, MT, BF], dt, tag="yshd", name="yshd2")
            y82 = y8_pool.tile([P, MT, BF], f8, tag="y8", name="y82")
            x2 = x_pool.tile([P, KT, BF], dt, tag="x", name="x2")
            # m-tiles 4-7 alias m-tiles 0-3's PSUM tags: the WAR fence keeps
            # the list scheduler from packing the m47 sweeps ahead of
            # hB x m03, which would delay the gather that gates step 3
            hA, hB = half_kts(0), half_kts(1)
            ypsum2 = {}
            for grp in range(2):
                for mt in range(4 * grp, 4 * grp + MH):
                    ypsum2[mt] = psum_pool.tile(
                        [P, BF], f32, tag=f"ps{mt % 4}", name=f"y2_{mt}"
                    )
                for kts, first, last in ((hA, True, False), (hB, False, True)):
                    for kt in kts:
                        for mt in range(4 * grp, 4 * grp + MH):
                            nc.tensor.matmul(
                                ypsum2[mt][:],
                                lhsT=lhsT_res(kt, mt),
                                rhs=x1[:, kt, :],
                                start=(first and kt == kts[0]),
                                stop=(last and kt == kts[-1]),
                            )
                for mt in range(4 * grp, 4 * grp + MH):
                    nc.vector.tensor_copy(yshd2[:, mt, :], ypsum2[mt][:])
                    nc.vector.tensor_copy(y82[:, mt, :], ypsum2[mt][:])
                gather_half(2, grp, y82, x2)
            transposes(2, yshd2, range(MT))

            # ---- step 3: interleaved column-half accumulation; one
            # stationary X2 k-tile feeds both 512-wide matmuls ----
            pt3 = proj_pool.tile([P, SH], dt, tag="pt3", name="pt3")
            pt[3] = pt3
            pp3 = [
                psum_pool.tile([P, 512], f32, tag=f"ps{4 * ns}", name=f"pp3_{ns}")
                for ns in range(2)
            ]
            order = hA + hB
            for ki, kt in enumerate(order):
                for ns in range(2):
                    nc.tensor.matmul(
                        pp3[ns][:],
                        lhsT=x2[:, kt, :],
                        rhs=lc_res[:, 2 * ns : 2 * ns + 2, kt, :],
                        start=(ki == 0),
                        stop=(ki == KT - 1),
                    )
            for ns in range(2):
                nc.vector.tensor_copy(
                    pt3[:, ns * 512 : (ns + 1) * 512], pp3[ns][:]
                )

            # ---- projection + output per column half ----
            for ns in range(2):
                for h in range(QH):
                    pp = psum_pool.tile(
                        [P, 512], f32, tag=f"ps{1 + h}", name=f"pp{ns}_{h}"
                    )
                    for j in range(K):
                        nc.tensor.matmul(
                            pp[:],
                            lhsT=whs[:, j, h, :],
                            rhs=pt[j][:, ns * 512 : (ns + 1) * 512],
                            start=(j == 0),
                            stop=(j == K - 1),
                        )
                    nc.vector.tensor_copy(out_sb[:, h, ns, :], pp[:])
                nc.sync.dma_start(
                    OUT.rearrange("h q (s n) -> q h s n", s=2)[:, :, ns, :],
                    out_sb[:, :, ns, :],
                )

    nc.compile()
    return nc


_CACHED = {}


def _get_nc(variant):
    if variant not in _CACHED:
        _CACHED[variant] = build_nc(variant)
    return _CACHED[variant]


def _prep_inputs(x, L, weight, variant):
    np_dt = _np_dt(variant)
    f32 = np.float32

    X0 = np.ascontiguousarray(
        x.astype(f32).transpose(1, 0, 2).reshape(N, BF)
    )  # [N, (b,fi)]
    X0_t = np.ascontiguousarray(
        X0.reshape(KT, P, BF).transpose(1, 0, 2)
    ).astype(np_dt)  # [P, KT, BF]
    W = weight.astype(f32)
    V = np.stack(
        [W[0] - W[2], W[1] - W[3], 2.0 * W[2], 2.0 * W[3]]
    )  # [4, F_IN, F_OUT]
    # block-diagonal packing: WH[p, j, h, bl*F_OUT+fo] = V[j,fi,fo]
    # for p == b*F_IN+fi, b == 2h + bl  (partition-major: contiguous DMA)
    WH = np.zeros((K, QH, BF, P), dtype=f32)
    for j in range(K):
        for b in range(B):
            h, bl = divmod(b, 2)
            WH[j, h, b * F_IN : (b + 1) * F_IN, bl * F_OUT : (bl + 1) * F_OUT] = V[j]
    WH = np.ascontiguousarray(WH.transpose(2, 0, 1, 3)).astype(np_dt)

    in_maps = []
    for r in range(NCORES):
        rows = slice(r * SH, (r + 1) * SH)
        Lc_r = np.ascontiguousarray(
            L[:, rows].reshape(KT, P, 4, SH4).transpose(2, 1, 0, 3)
        ).astype(np_dt)  # [4, P, KT, SH4]
        X0T_r = np.ascontiguousarray(X0[rows, :].T).astype(np_dt)
        in_maps.append({"Lc": Lc_r, "X0": X0_t, "X0T": X0T_r, "WH": WH})
    return in_maps


def _assemble(results, bias):
    out = np.empty((B, N, F_OUT), dtype=np.float32)
    for r in range(NCORES):
        outT = results[r]["OUT"].astype(np.float32)  # [QH, 128, SH]
        for b in range(B):
            h, bl = divmod(b, 2)
            out[b, r * SH : (r + 1) * SH, :] = outT[
                h, bl * F_OUT : (bl + 1) * F_OUT, :
            ].T
    out += bias.astype(np.float32)
    return out


def run(x, L, weight, bias, variant=VARIANT, trace=False):
    nc = _get_nc(variant)
    in_maps = _prep_inputs(x, L, weight, variant)
    last_err = None
    for attempt in range(3):
        try:
            res = bass_utils.run_bass_kernel_spmd(
                nc,
                in_maps,
                core_ids=list(range(NCORES)),
                trace=trace,
                trace_cores=list(range(NCORES)) if trace else None,
            )
            break
        except Exception as e:  # transient device wedge: retry
            last_err = e
            import time

            time.sleep(10)
    else:
        raise last_err
    out = _assemble(res.results, bias)
    return out, res


def kernel(x, L, weight, bias):
    out, _ = run(
        np.asarray(x), np.asarray(L), np.asarray(weight), np.asarray(bias)
    )
    return out
